# revision 48
# baseline (speedup 1.0000x reference)
"""GQA attention block (B=2, N=2048, D=2048, Hq=32, Hkv=8, d=64) on 8 TRN2 NeuronCores.

Sharding: core c = b*4 + hg  (data-parallel over batch b in {0,1}; tensor-parallel
over 4 head-groups hg, each owning 8 q-heads / 2 kv-heads).  Each core computes a
row-parallel partial of the output projection for its batch; the host sums the 4
partials per batch (fp16 partials).

All matmuls run in bf16 (fp32 matmul costs 4 PE cycles/row vs 1 for bf16);
PSUM accumulation stays fp32 and softmax exp reads fp32 PSUM scores.

Key scheduling structure:
 - Head pairs are (one q-head on kv0, one q-head on kv1) via a host-side
   weight permutation, so the two 64-partition PE row groups read the two
   halves of a single feature-major K tile (no partition-swapped K copy)
   and their score matmuls run CONCURRENTLY.
 - q/k transposes to feature-major go through the DMA XBAR
   (dma_start_transpose), not the PE; all input loads are issued up front
   as wait-free DMAs so the transposes never head-of-line-block a load on
   the in-order sync queue.
 - The whole kernel runs in ONE ScalarE activation table set
   (natural_log_exp_and_others): rmsnorm's rsqrt is computed as
   exp(-0.5*ln(ms+eps)), so Exp can be freely interleaved anywhere.
 - q-chunk 0 (all-diagonal, 16 k-tiles) has its scores+exp PREFUSED into
   phase 1's spare ScalarE/PE slack; phase 2 then opens with a dense PV
   avalanche instead of an exp-bound fragmented stream.
 - Remaining chunks run ascending; each finished chunk's out-projection
   groups are flushed into the next chunk's exp-bound kt pipeline to fill
   tensor-engine gaps (in-order queues: the filler must sit between the
   stalls).  PV runs as one 1024-wide bf16 matmul per k-tile (V with an
   appended ones-column produces y plus the softmax denominator).
"""

import numpy as np

D_MODEL = 2048
H_Q, H_KV, D_HEAD = 32, 8, 64
B = 2
N = 2048
ROPE_BASE = 10000.0
EPS = 1e-6
NCORES = 8
P = 128


def _modules():
    import sys

    for p in ("/opt/trn_rl_repo",):
        if p not in sys.path:
            sys.path.insert(0, p)
    import concourse.bass as bass
    import concourse.tile as tile
    from concourse import bacc, mybir

    return bass, tile, bacc, mybir


def build_nc(n_tok=N, causal=True):
    """Build the single-core SPMD Bass program (identical on all 8 cores)."""
    from contextlib import ExitStack

    bass, tile, bacc, mybir = _modules()
    f32 = mybir.dt.float32
    f16 = mybir.dt.float16
    bf16 = mybir.dt.bfloat16
    ts = bass.ts
    AF = mybir.ActivationFunctionType
    OP = mybir.AluOpType

    NT = n_tok // P           # token tiles
    DC = D_MODEL // P         # contraction chunks for qkv proj
    QC = n_tok // 512         # query chunks of 512
    NG = n_tok // 512         # x-load groups (512 tokens each)
    assert causal and QC == 4 and n_tok % 512 == 0

    nc = bacc.Bacc("TRN2", target_bir_lowering=False, debug=False,
                   num_devices=NCORES)

    # host-pre-tiled layouts: every DMA reads multi-KB contiguous runs per
    # partition (512B-run gathers cost ~48ns of DMA-engine time per
    # descriptor and starve the XBAR transposes via the completion ring)
    xT = nc.dram_tensor("xT", [P, n_tok // 256, DC, 256], bf16,
                        kind="ExternalInput").ap()
    wqkv = nc.dram_tensor("wqkv", [P, DC, 768], bf16,
                          kind="ExternalInput").ap()
    wo = nc.dram_tensor("wo", [P, 4, D_MODEL], bf16,
                        kind="ExternalInput").ap()
    tabq = nc.dram_tensor("tabq", [P, NT, 4, 32], bf16, kind="ExternalInput").ap()
    tabk = nc.dram_tensor("tabk", [P, NT, 4, 32], bf16, kind="ExternalInput").ap()
    out = nc.dram_tensor("out", [n_tok, D_MODEL], f16, kind="ExternalOutput").ap()

    with ExitStack() as ctx:
        tc = ctx.enter_context(tile.TileContext(nc))

        cpool = ctx.enter_context(tc.tile_pool(name="const", bufs=1))
        # persistent activations (all bf16: they feed matmuls)
        # qkfm[qc][:, c, :] = feature-major q (pair c<4) / k (c=4,
        # [kv0|kv1] on partitions) for 512-token chunk qc — ONE XBAR
        # dma-transpose per token tile writes all five 128-feature groups.
        # Chunked tiles keep reader deps fine-grained: chunk-1 scores wait
        # only on chunk-1's four XBARs, not all sixteen.
        qkfm = [cpool.tile([P, 5, 512], bf16, name=f"qkfm{i}")
                for i in range(QC)]
        yfm = [cpool.tile([P, n_tok], bf16, name=f"yfm{c}") for c in range(4)]
        vsb = [cpool.tile([P, 130], bf16, name=f"vsb{t}") for t in range(NT)]
        eps_t = cpool.tile([P, 1], f32, name="eps_t")
        nc.gpsimd.memset(eps_t[:], EPS)
        # preload the GpSimd broadcast library at t~0: its first real use
        # (normalize after the avalanche) would otherwise trigger a ~7us
        # UNLOAD_LIB/LOAD_LIB swap on the phase-transition critical path
        libwarm = cpool.tile([2, 1], f32, name="libwarm")
        nc.gpsimd.partition_broadcast(libwarm[:], eps_t[0:1, 0:1],
                                      channels=2)
        for t in range(NT):
            nc.gpsimd.memset(vsb[t][:, 64:65], 1.0)
            nc.gpsimd.memset(vsb[t][:, 129:130], 1.0)
        # prefused exp(scores) of q-chunk 0: [pair c, diag k-tile j]
        egpre = ctx.enter_context(tc.tile_pool(name="egpre", bufs=1))
        eg0 = [egpre.tile([P, 1024], bf16, name=f"eg0_{i}") for i in range(16)]

        # scores psum ring, shared by the phase-1 prefuse, phase-2 kt
        # pipeline, and the out-projection groups
        s_ps = ctx.enter_context(
            tc.tile_pool(name="spsum", bufs=1, space="PSUM"))
        s_ctr = [0]

        s_seq = [0]

        def s_tile():
            # alternate tags: dependency tracking is tag-coarse, so two
            # tags of one buf each give the same double-buffering with
            # waits that only cover the actual slot being reused
            s_ctr[0] ^= 1
            s_seq[0] += 1
            return s_ps.tile([P, 1024], f32, tag=f"s{s_ctr[0]}",
                             name=f"s_{s_seq[0]}")

        def emit_scores_exp(qc, c, kt, eg, nkt):
            """Scores for (q-chunk qc, pair c, k-tile kt) -> exp into eg."""
            jl = kt - 4 * qc  # >=0 inside the diagonal quad
            diag = jl >= 0
            o = 128 * jl if diag else 0
            ps_s = s_tile()
            kf = qkfm[kt // 4][:, 4, :]
            qf = qkfm[qc]
            nc.tensor.matmul(
                ps_s[:, o:512],
                kf[0:64, ts(kt % 4, P)],
                qf[0:64, c, o:512],
                start=True, stop=True)
            nc.tensor.matmul(
                ps_s[:, 512 + o:1024],
                kf[64:128, ts(kt % 4, P)],
                qf[64:128, c, o:512],
                start=True, stop=True)
            if not diag:
                nc.scalar.activation(eg[:], ps_s[:], AF.Exp)
            else:
                nc.scalar.activation(
                    eg[:].rearrange("p (j q) -> p j q", q=512)[:, :, o:512],
                    ps_s[:].rearrange("p (j q) -> p j q", q=512)[:, :, o:512],
                    AF.Exp)
                # causal triangle at the diagonal 128-col block
                nc.gpsimd.affine_select(
                    eg[:].rearrange("p (j q) -> p j q", q=512)[:, :, o:o + 128],
                    eg[:].rearrange("p (j q) -> p j q", q=512)[:, :, o:o + 128],
                    pattern=[[0, 2], [1, 128]],
                    compare_op=OP.is_ge,
                    fill=0.0,
                    base=0,
                    channel_multiplier=-1)

        # ---------------- phase 1: qkv + norm + rope + transpose ----------
        with ExitStack() as p1:
            wpool = p1.enter_context(tc.tile_pool(name="wqkv", bufs=1))
            tpool = p1.enter_context(tc.tile_pool(name="tabs", bufs=1))
            XB = 2 * NG       # ALL x groups resident: every load DMA is
            # issued up front, so the XBAR transposes (emitted per tile)
            # only ever ring-wait on loads that completed long ago
            xpool = p1.enter_context(tc.tile_pool(name="xg", bufs=XB))
            # bufs=4: four token tiles in flight so each tile's serial
            # rmsnorm/rope chain + XBAR transpose DMA overlaps the next
            # tiles' matmuls
            wkk = p1.enter_context(tc.tile_pool(name="qkvwork", bufs=4))
            qkv_ps = p1.enter_context(
                tc.tile_pool(name="qkvpsum", bufs=2, space="PSUM"))

            # ALL input loads are issued up front as pure (wait-free) DMAs
            # so nothing with a runtime dependency (the XBAR transposes)
            # ever head-of-line-blocks a load on the in-order sync queue
            xgs = {}

            def issue_xg(g, split=1):
                xgs[g] = xpool.tile([P, DC, 256], bf16, tag="xg",
                                    name=f"xg{g}")
                src = xT[:, g]
                step = DC // split
                for s in range(split):
                    sl = slice(s * step, (s + 1) * step)
                    nc.sync.dma_start(xgs[g][:, sl], src[:, sl])

            # tile-0's inputs are issued interleaved in dc-consumption
            # order (x chunk s feeds dc 4s..4s+3, which read wq tile s)
            # so the first qkv chain never outruns the DMA stream
            wq_sb = [wpool.tile([P, 4, 768], bf16, name=f"wq{i}")
                     for i in range(4)]
            xgs[0] = xpool.tile([P, DC, 256], bf16, tag="xg", name="xg0")
            x0src = xT[:, 0]
            for s in range(4):
                sl = slice(4 * s, 4 * s + 4)
                nc.sync.dma_start(xgs[0][:, sl], x0src[:, sl])
                for j in range(4):
                    nc.sync.dma_start(wq_sb[s][:, j:j + 1],
                                      wqkv[:, 4 * s + j:4 * s + j + 1, :])
            issue_xg(1)

            tq = tpool.tile([P, NT, 4, 32], bf16)
            nc.sync.dma_start(tq[:], tabq)
            tk = tpool.tile([P, NT, 4, 32], bf16)
            nc.sync.dma_start(tk[:], tabk)
            for g in range(2, 2 * NG):
                issue_xg(g)

            # q-chunk-0 prefuse schedule: 2 (pair, k-tile) units after each
            # token tile from tile 6 on (their q/k XBAR tiles 0-3 are long
            # since landed; ScalarE has slack under the qkv matmul stream)
            prefuse = [(c, j) for c in range(4) for j in range(4)]
            pf = [0]

            def emit_prefuse(k=2):
                for _ in range(k):
                    if pf[0] < len(prefuse):
                        c, j = prefuse[pf[0]]
                        pf[0] += 1
                        emit_scores_exp(0, c, j, eg0[4 * c + j][:], 4)

            for g in range(2 * NG):
                xg = xgs[g]
                for lt in range(2):
                    tt = g * 2 + lt
                    ps = qkv_ps.tile([P, 768], f32, tag="qkv")
                    for dc in range(DC):
                        lhsT = xg[:, dc, ts(lt, P)]
                        wsl = wq_sb[dc // 4][:, dc % 4]
                        nc.tensor.matmul(ps[:, 0:512], lhsT, wsl[:, 0:512],
                                         start=(dc == 0), stop=(dc == DC - 1))
                        nc.tensor.matmul(ps[:, 512:768], lhsT, wsl[:, 512:768],
                                         start=(dc == 0), stop=(dc == DC - 1))
                    # --- rmsnorm: rs = exp(-0.5*ln(ms+eps)); Square/Ln/Exp
                    # all live in one ACT table set, so no table reloads ---
                    sq = wkk.tile([P, 640], f32, tag="sq")
                    nc.scalar.activation(sq[:], ps[:, 0:640], AF.Square)
                    ssq = wkk.tile([P, 10], f32, tag="ssq")
                    nc.vector.reduce_sum(
                        ssq[:], sq[:].rearrange("p (h d) -> p h d", d=64),
                        axis=mybir.AxisListType.X)
                    lg = wkk.tile([P, 10], f32, tag="lg")
                    nc.scalar.activation(lg[:], ssq[:], AF.Ln,
                                         bias=eps_t[:], scale=1.0 / 64)
                    rs = wkk.tile([P, 10], f32, tag="rs")
                    nc.scalar.activation(rs[:], lg[:], AF.Exp, scale=-0.5)
                    qn = wkk.tile([P, 512], bf16, tag="qn")
                    nc.vector.tensor_tensor(
                        qn[:].rearrange("p (h d) -> p h d", d=64),
                        ps[:, 0:512].rearrange("p (h d) -> p h d", d=64),
                        rs[:, 0:8, None].to_broadcast([P, 8, 64]), OP.mult)
                    kn = wkk.tile([P, 128], bf16, tag="kn")
                    nc.vector.tensor_tensor(
                        kn[:].rearrange("p (h d) -> p h d", d=64),
                        ps[:, 512:640].rearrange("p (h d) -> p h d", d=64),
                        rs[:, 8:10, None].to_broadcast([P, 2, 64]), OP.mult)
                    # --- v copy on the DVE (ones cols at 64/129) ---
                    nc.vector.tensor_copy(
                        vsb[tt][:, 0:130].rearrange(
                            "p (j q) -> p j q", q=65)[:, :, 0:64],
                        ps[:, 640:768].rearrange("p (j q) -> p j q", q=64))
                    # --- rope: 3 DVE ops per tensor via host-folded tables
                    # tab rows are [A, B, C, -D]; viewed as [P, 2, 2, 32] the
                    # pairs are (A,C) and (B,-D), so
                    # dv = t1*(A,C) - t2*(B,-D) = (t1*A - t2*B | t1*C + t2*D)
                    qkr = wkk.tile([P, 640], bf16, tag="qkr")
                    for (src, dsl, tab, nh) in ((qn, slice(0, 512), tq, 8),
                                                (kn, slice(512, 640), tk, 2)):
                        sv = src[:].rearrange("p (h d) -> p h d", d=64)
                        dv = qkr[:, dsl].rearrange("p (h two f) -> p h two f",
                                                   two=2, f=32)
                        tabv = tab[:, tt].rearrange("p (g two) f -> p two g f",
                                                    two=2)
                        t1 = sv[:, :, None, 0:32].to_broadcast([P, nh, 2, 32])
                        t2 = sv[:, :, None, 32:64].to_broadcast([P, nh, 2, 32])
                        AC = tabv[:, 0:1, :, :].to_broadcast([P, nh, 2, 32])
                        BD = tabv[:, 1:2, :, :].to_broadcast([P, nh, 2, 32])
                        u13 = wkk.tile([P, nh, 2, 32], bf16, tag=f"u13_{nh}")
                        u24 = wkk.tile([P, nh, 2, 32], bf16, tag=f"u24_{nh}")
                        nc.vector.tensor_tensor(u13[:], t1, AC, OP.mult)
                        nc.vector.tensor_tensor(u24[:], t2, BD, OP.mult)
                        nc.vector.tensor_tensor(dv, u13[:], u24[:],
                                                OP.subtract)
                    # --- transpose to feature-major via the DMA XBAR:
                    # out[p, g, t] = qkr[t, 128g + p], one dma for q AND k
                    # (keeps the in-order sync queue's descriptor-gen time
                    # per tile low enough to track the rope stream) ---
                    nc.sync.dma_start_transpose(
                        qkfm[tt // 4][:, :, ts(tt % 4, P)], qkr[:])
                    if tt >= 8:
                        emit_prefuse(3)
            emit_prefuse(len(prefuse))  # safety: should be drained already

        # ---------------- phase 2: attention + out projection ------------
        wopool = ctx.enter_context(tc.tile_pool(name="wo", bufs=1))
        wo_sb = wopool.tile([P, 4, D_MODEL], bf16, name="wo_sb")
        nc.sync.dma_start(wo_sb[:], wo)
        with ExitStack() as p2:
            epool = p2.enter_context(tc.tile_pool(name="exp", bufs=4))
            npool = p2.enter_context(tc.tile_pool(name="nrm", bufs=2))
            opool = p2.enter_context(tc.tile_pool(name="osb", bufs=3))
            y_ps = p2.enter_context(
                tc.tile_pool(name="ypsum", bufs=1, space="PSUM"))
            y_ctr = [0]

            y_seq = [0]

            def y_tile():
                y_ctr[0] ^= 1
                y_seq[0] += 1
                return y_ps.tile([65, 1024], f32, tag=f"y{y_ctr[0]}",
                                 name=f"y_{y_seq[0]}")

            pending = []          # (token tile, output half) groups
            flush_ctr = [0]
            OG_INLINE = True      # inline out-proj at sustainable cadence

            def emit_ogroup(cast_on_scalar=False):
                t, og = pending.pop(0)
                ps_o = s_tile()
                for oc2 in range(2):
                    for yc in range(4):
                        nc.tensor.matmul(
                            ps_o[:, ts(oc2, 512)],
                            yfm[yc][:, ts(t, P)],
                            wo_sb[:, yc, 1024 * og + 512 * oc2:
                                  1024 * og + 512 * (oc2 + 1)],
                            start=(yc == 0), stop=(yc == 3))
                ob = opool.tile([P, 1024], f16, tag="ob")
                if cast_on_scalar:
                    # drain: exps are over, ScalarE is free, and the DVE is
                    # busy with the final normalize chains
                    nc.scalar.activation(ob[:], ps_o[:], AF.Copy)
                else:
                    nc.vector.tensor_copy(ob[:], ps_o[:])
                nc.sync.dma_start(out[ts(t, P), ts(og, 1024)], ob[:])

            def emit_pv(kt, eg, o, ps_y, nkt):
                nc.tensor.matmul(
                    ps_y[:, o:512], vsb[kt][:, 0:65], eg[:, o:512],
                    start=(kt == 0), stop=(kt == nkt - 1))
                nc.tensor.matmul(
                    ps_y[:, 512 + o:1024], vsb[kt][:, 65:130],
                    eg[:, 512 + o:1024],
                    start=(kt == 0), stop=(kt == nkt - 1))

            def normalize(ps_y, c, qc):
                # 1/den via DVE recip + GpSimd partition broadcast
                # (recip can't read PSUM; vector copy bounces row 64)
                draw = npool.tile([1, 1024], f32, tag="draw")
                nc.vector.tensor_copy(draw[0:1, :], ps_y[64:65, :])
                rec = npool.tile([1, 1024], f32, tag="rec")
                nc.vector.reciprocal_approx_fast(rec[0:1, :], draw[0:1, :])
                rexp = npool.tile([64, 1024], f32, tag="rexp")
                nc.gpsimd.partition_broadcast(rexp[:], rec[0:1, :],
                                              channels=64)
                nc.vector.tensor_tensor(yfm[c][0:64, ts(qc, 512)],
                                        ps_y[0:64, 0:512],
                                        rexp[:, 0:512], OP.mult)
                nc.vector.tensor_tensor(yfm[c][64:128, ts(qc, 512)],
                                        ps_y[0:64, 512:1024],
                                        rexp[:, 512:1024], OP.mult)

            # `held` carries the not-yet-emitted PV (+ pair finalizer) of the
            # previous k-tile ACROSS pair boundaries, so scores/exp of the
            # next pair keep both engines fed while the last PV of the
            # previous pair waits on its exp semaphore.
            held = []   # FIFO of (pv_fn, final_fn or None); depth-2 skew

            # --- q-chunk 0: dense PV avalanche over the prefused exps ---
            # pairs 0/1 emitted directly; pairs 2/3 seeded into `held` so
            # their PVs (which wait on the y-ring released by the pair-0/1
            # normalize chains) interleave with qc1's first scores instead
            # of blocking the in-order tensor queue
            def avalanche(c):
                ps_y = y_tile()
                for j in range(4):
                    emit_pv(j, eg0[4 * c + j][:], 128 * j, ps_y, 4)
                normalize(ps_y, c, 0)

            avalanche(0)
            avalanche(1)
            for c in (2, 3):
                held.append((lambda c=c: avalanche(c), None))
            for tl in range(4):
                for og in range(2):
                    pending.append((tl, og))

            def flush_held():
                if not held:
                    return
                pv_fn, final_fn = held.pop(0)
                pv_fn()
                if final_fn is not None:
                    final_fn()
                flush_ctr[0] += 1
                # don't queue an ogroup cast on the DVE right before a
                # pair finalize: the cast would delay the normalize mults
                # that release the y-ring for the pair after next
                imminent = held and held[0][1] is not None
                if (OG_INLINE and pending and not imminent
                        and flush_ctr[0] >= 3 and flush_ctr[0] % 4 == 0):
                    emit_ogroup()

            for qc in range(1, QC):
                for c in range(4):
                    nkt = 4 * qc + 4
                    ps_y = y_tile()

                    for kt in range(nkt):
                        jl = kt - 4 * qc
                        diag = jl >= 0
                        o = 128 * jl if diag else 0
                        eg = epool.tile([P, 1024], bf16, tag="eg")
                        emit_scores_exp(qc, c, kt, eg[:], nkt)
                        if len(held) >= 2:
                            flush_held()
                        is_last = kt == nkt - 1
                        held.append((
                            lambda kt=kt, eg=eg, o=o, ps_y=ps_y, nkt=nkt:
                                emit_pv(kt, eg[:], o, ps_y, nkt),
                            (lambda ps_y=ps_y, c=c, qc=qc:
                                normalize(ps_y, c, qc)) if is_last else None))
                # queue this q-chunk's out-projection groups (flushed during
                # the next chunk; the final chunk's groups flush below)
                for tl in range(4):
                    for og in range(2):
                        pending.append((4 * qc + tl, og))
            while held:
                flush_held()
            while pending:
                emit_ogroup(cast_on_scalar=True)

    # Force a single ScalarE activation table: natural_log_exp_and_others
    # holds every func this kernel uses (Exp, Ln, Square, Copy).  The
    # greedy per-function chooser would otherwise ping-pong between the
    # exp and natural-log sets (1.28us per ACT_TABLE_LOAD).  Emptying the
    # other sets (names/indices preserved, so act_func_set_id stays valid)
    # makes the covering analysis land every activation on the one set.
    from concourse import hw_specs
    tables = hw_specs.get_activation_tables(nc.m.arch)
    for name, funcs in tables.items():
        if name != "natural_log_exp_and_others":
            funcs.clear()
    nc.compile()
    return nc


def _rope_tables(pos, norm_w, scale):
    """Build [P, NT, 4, 32] tables A,B,C,D for out1 = t1*A - t2*B,
    out2 = t1*C + t2*D (NeoX rope with folded norm weight + score scale)."""
    n_tok = pos.shape[0]
    f = np.arange(0, D_HEAD, 2, dtype=np.float64) / D_HEAD
    inv_freq = 1.0 / (ROPE_BASE ** f)                       # [32]
    ang = pos.astype(np.float64)[:, None] * inv_freq[None, :]  # [n, 32]
    cos, sin = np.cos(ang), np.sin(ang)
    w1 = norm_w[:32].astype(np.float64)
    w2 = norm_w[32:].astype(np.float64)
    A = cos * w1 * scale
    Bt = sin * w2 * scale
    C = sin * w1 * scale
    D = cos * w2 * scale
    # D negated: the kernel computes t1*(A,C) - t2*(B,-D) in two fused ops
    tab = np.stack([A, Bt, C, -D], axis=1).astype(np.float32)  # [n, 4, 32]
    return np.ascontiguousarray(
        tab.reshape(n_tok // P, P, 4, 32).transpose(1, 0, 2, 3))


def make_in_maps(x, pos, qkv_w, out_w, q_norm_w, k_norm_w, n_tok=N):
    import ml_dtypes
    bf16 = ml_dtypes.bfloat16

    scale = D_HEAD ** -0.5
    tabq = _rope_tables(pos, q_norm_w, scale).astype(bf16)
    tabk = _rope_tables(pos, k_norm_w, 1.0).astype(bf16)
    wq_all = qkv_w[0:H_Q * D_HEAD].reshape(H_Q, D_HEAD, D_MODEL)
    wk_all = qkv_w[H_Q * D_HEAD:(H_Q + H_KV) * D_HEAD].reshape(
        H_KV, D_HEAD, D_MODEL)
    wv_all = qkv_w[(H_Q + H_KV) * D_HEAD:].reshape(H_KV, D_HEAD, D_MODEL)
    wo_all = out_w.reshape(D_MODEL, H_Q, D_HEAD)

    in_maps = []
    for c in range(NCORES):
        b, hg = divmod(c, 4)
        # head order [0,4,1,5,2,6,3,7]: pair i = (head on kv0, head on kv1)
        # so the kernel's two 64-partition row groups use kfm's two halves
        # directly (no partition-swapped K copy)
        heads = [8 * hg + (i % 2) * 4 + i // 2 for i in range(8)]
        kvs = [2 * hg, 2 * hg + 1]
        wsel = np.concatenate([
            wq_all[heads].reshape(512, D_MODEL),
            wk_all[kvs].reshape(128, D_MODEL),
            wv_all[kvs].reshape(128, D_MODEL)], axis=0)    # [768, D]
        # pre-tiled: xT[p, g, o, t], wqkv[p, o, r], wo[p, o, d] — see the
        # dram_tensor declarations in build_nc
        xt = np.ascontiguousarray(
            x[b].T.reshape(16, P, n_tok // 256, 256).transpose(1, 2, 0, 3)
        ).astype(bf16)
        wq_t = np.ascontiguousarray(
            wsel.T.reshape(16, P, 768).transpose(1, 0, 2)).astype(bf16)
        wo_t = np.ascontiguousarray(
            wo_all[:, heads].reshape(D_MODEL, 512).T
            .reshape(4, P, D_MODEL).transpose(1, 0, 2)).astype(bf16)
        in_maps.append({
            "xT": xt,
            "wqkv": wq_t,
            "wo": wo_t,
            "tabq": tabq,
            "tabk": tabk,
        })
    return in_maps


def _reference_host(x, mask, pos, qkv_w, out_w, q_norm_w, k_norm_w):
    """Pure-numpy fallback, used only if the mask is not causal."""
    xx = x.astype(np.float64)
    qkv = xx @ qkv_w.T.astype(np.float64)
    Bsz, Nl, _ = x.shape
    qkv = qkv.reshape(Bsz, Nl, H_Q + 2 * H_KV, D_HEAD).transpose(0, 2, 1, 3)
    q, k, v = (qkv[:, :H_Q], qkv[:, H_Q:H_Q + H_KV], qkv[:, H_Q + H_KV:])

    def rms(t, w):
        var = np.mean(t * t, axis=-1, keepdims=True)
        return t / np.sqrt(var + EPS) * w

    def rope(t):
        f = np.arange(0, D_HEAD, 2) / D_HEAD
        inv = 1.0 / (ROPE_BASE ** f)
        ang = pos.astype(np.float64)[:, None] * inv[None, :]
        cs, sn = np.cos(ang), np.sin(ang)
        t1, t2 = t[..., :32], t[..., 32:]
        return np.concatenate([t1 * cs - t2 * sn, t1 * sn + t2 * cs], axis=-1)

    q, k = rope(rms(q, q_norm_w)), rope(rms(k, k_norm_w))
    qg = q.reshape(Bsz, H_KV, 4, Nl, D_HEAD)
    sc = np.einsum("bhgnd,bhmd->bhgnm", qg, k) * (D_HEAD ** -0.5)
    sc = np.where(mask[None, None, None], -np.inf, sc)
    sc -= sc.max(axis=-1, keepdims=True)
    p = np.exp(sc)
    p /= p.sum(axis=-1, keepdims=True)
    y = np.einsum("bhgnm,bhmd->bhgnd", p, v)
    y = y.reshape(Bsz, H_Q, Nl, D_HEAD).transpose(0, 2, 1, 3).reshape(
        Bsz, Nl, D_MODEL)
    return (y @ out_w.T.astype(np.float64)).astype(np.float32)


_NC_CACHE = {}


def run_on_device(in_maps, n_tok=N, trace=False, trace_kwargs=None):
    import sys
    for p in ("/opt/trn_rl_repo",):
        if p not in sys.path:
            sys.path.insert(0, p)
    from concourse.bass_utils import run_bass_kernel_spmd

    key = n_tok
    if key not in _NC_CACHE:
        _NC_CACHE[key] = build_nc(n_tok)
    nc = _NC_CACHE[key]
    return run_bass_kernel_spmd(
        nc, in_maps, list(range(len(in_maps))), trace=trace,
        **(trace_kwargs or {}))


def kernel(x, mask, pos, qkv_w, out_w, q_norm_w, k_norm_w):
    x = np.asarray(x, dtype=np.float32)
    mask = np.asarray(mask)
    pos = np.asarray(pos)
    causal = bool(
        np.array_equal(mask,
                       np.triu(np.ones((N, N), dtype=bool), k=1)))
    if not causal:
        return _reference_host(x, mask, pos, np.asarray(qkv_w),
                               np.asarray(out_w), np.asarray(q_norm_w),
                               np.asarray(k_norm_w))
    in_maps = make_in_maps(x, pos, np.asarray(qkv_w, dtype=np.float32),
                           np.asarray(out_w, dtype=np.float32),
                           np.asarray(q_norm_w, dtype=np.float32),
                           np.asarray(k_norm_w, dtype=np.float32))
    res = run_on_device(in_maps)
    outs = [r["out"].astype(np.float32) for r in res.results]
    full = np.empty((B, N, D_MODEL), dtype=np.float32)
    for b in range(B):
        full[b] = outs[4 * b] + outs[4 * b + 1] + outs[4 * b + 2] + outs[4 * b + 3]
    return full


# revision 50
# speedup vs baseline: 1.0534x; 1.0534x over previous
"""GQA attention block (B=2, N=2048, D=2048, Hq=32, Hkv=8, d=64) on 8 TRN2 NeuronCores.

Sharding: core c = b*4 + hg  (data-parallel over batch b in {0,1}; tensor-parallel
over 4 head-groups hg, each owning 8 q-heads / 2 kv-heads).  Each core computes a
row-parallel partial of the output projection for its batch; the host sums the 4
partials per batch (fp16 partials).

All matmuls run in bf16 (fp32 matmul costs 4 PE cycles/row vs 1 for bf16);
PSUM accumulation stays fp32 and softmax exp reads fp32 PSUM scores.

Key scheduling structure:
 - Head pairs are (one q-head on kv0, one q-head on kv1) via a host-side
   weight permutation, so the two 64-partition PE row groups read the two
   halves of a single feature-major K tile (no partition-swapped K copy)
   and their score matmuls run CONCURRENTLY.
 - q/k transposes to feature-major go through the DMA XBAR
   (dma_start_transpose), not the PE; all input loads are issued up front
   as wait-free DMAs so the transposes never head-of-line-block a load on
   the in-order sync queue.
 - The whole kernel runs in ONE ScalarE activation table set
   (natural_log_exp_and_others): rmsnorm's rsqrt is computed as
   exp(-0.5*ln(ms+eps)), so Exp can be freely interleaved anywhere.
 - q-chunk 0 (all-diagonal, 16 k-tiles) has its scores+exp PREFUSED into
   phase 1's spare ScalarE/PE slack; phase 2 then opens with a dense PV
   avalanche instead of an exp-bound fragmented stream.
 - Remaining chunks run ascending; each finished chunk's out-projection
   groups are flushed into the next chunk's exp-bound kt pipeline to fill
   tensor-engine gaps (in-order queues: the filler must sit between the
   stalls).  PV runs as one 1024-wide bf16 matmul per k-tile (V with an
   appended ones-column produces y plus the softmax denominator).
"""

import numpy as np

D_MODEL = 2048
H_Q, H_KV, D_HEAD = 32, 8, 64
B = 2
N = 2048
ROPE_BASE = 10000.0
EPS = 1e-6
NCORES = 8
P = 128


def _modules():
    import sys

    for p in ("/opt/trn_rl_repo",):
        if p not in sys.path:
            sys.path.insert(0, p)
    import concourse.bass as bass
    import concourse.tile as tile
    from concourse import bacc, mybir

    return bass, tile, bacc, mybir


def build_nc(n_tok=N, causal=True):
    """Build the single-core SPMD Bass program (identical on all 8 cores)."""
    from contextlib import ExitStack

    bass, tile, bacc, mybir = _modules()
    f32 = mybir.dt.float32
    f16 = mybir.dt.float16
    bf16 = mybir.dt.bfloat16
    ts = bass.ts
    AF = mybir.ActivationFunctionType
    OP = mybir.AluOpType

    NT = n_tok // P           # token tiles
    DC = D_MODEL // P         # contraction chunks for qkv proj
    QC = n_tok // 512         # query chunks of 512
    NG = n_tok // 512         # x-load groups (512 tokens each)
    assert causal and QC == 4 and n_tok % 512 == 0

    nc = bacc.Bacc("TRN2", target_bir_lowering=False, debug=False,
                   num_devices=NCORES)

    # host-pre-tiled layouts: every DMA reads multi-KB contiguous runs per
    # partition (512B-run gathers cost ~48ns of DMA-engine time per
    # descriptor and starve the XBAR transposes via the completion ring)
    xT = nc.dram_tensor("xT", [P, n_tok // 256, DC, 256], bf16,
                        kind="ExternalInput").ap()
    wqkv = nc.dram_tensor("wqkv", [P, DC, 768], bf16,
                          kind="ExternalInput").ap()
    wo = nc.dram_tensor("wo", [P, 4, D_MODEL], bf16,
                        kind="ExternalInput").ap()
    tabq = nc.dram_tensor("tabq", [P, NT, 4, 32], bf16, kind="ExternalInput").ap()
    tabk = nc.dram_tensor("tabk", [P, NT, 4, 32], bf16, kind="ExternalInput").ap()
    out = nc.dram_tensor("out", [n_tok, D_MODEL], f16, kind="ExternalOutput").ap()

    with ExitStack() as ctx:
        tc = ctx.enter_context(tile.TileContext(nc))

        cpool = ctx.enter_context(tc.tile_pool(name="const", bufs=1))
        # persistent activations (all bf16: they feed matmuls)
        # qkfm[qc][:, c, :] = feature-major q (pair c<4) / k (c=4,
        # [kv0|kv1] on partitions) for 512-token chunk qc — ONE XBAR
        # dma-transpose per token tile writes all five 128-feature groups.
        # Chunked tiles keep reader deps fine-grained: chunk-1 scores wait
        # only on chunk-1's four XBARs, not all sixteen.
        qkfm = [cpool.tile([P, 5, 512], bf16, name=f"qkfm{i}")
                for i in range(QC)]
        yfm = [cpool.tile([P, n_tok], bf16, name=f"yfm{c}") for c in range(4)]
        vsb = [cpool.tile([P, 130], bf16, name=f"vsb{t}") for t in range(NT)]
        eps_t = cpool.tile([P, 1], f32, name="eps_t")
        nc.gpsimd.memset(eps_t[:], EPS)
        # preload the GpSimd broadcast library at t~0: its first real use
        # (normalize after the avalanche) would otherwise trigger a ~7us
        # UNLOAD_LIB/LOAD_LIB swap on the phase-transition critical path
        libwarm = cpool.tile([2, 1], f32, name="libwarm")
        nc.gpsimd.partition_broadcast(libwarm[:], eps_t[0:1, 0:1],
                                      channels=2)
        for t in range(NT):
            nc.gpsimd.memset(vsb[t][:, 64:65], 1.0)
            nc.gpsimd.memset(vsb[t][:, 129:130], 1.0)
        # prefused exp(scores): q-chunk 0 (all four pairs, idx 4c+j) and
        # q-chunk 1 pairs 0/1 (idx 16 + 8c + kt)
        egpre = ctx.enter_context(tc.tile_pool(name="egpre", bufs=1))
        eg0 = [egpre.tile([P, 1024], bf16, name=f"eg0_{i}") for i in range(32)]

        # scores psum ring, shared by the phase-1 prefuse, phase-2 kt
        # pipeline, and the out-projection groups
        s_ps = ctx.enter_context(
            tc.tile_pool(name="spsum", bufs=1, space="PSUM"))
        s_ctr = [0]

        s_seq = [0]

        def s_tile():
            # alternate tags: dependency tracking is tag-coarse, so two
            # tags of one buf each give the same double-buffering with
            # waits that only cover the actual slot being reused
            s_ctr[0] ^= 1
            s_seq[0] += 1
            return s_ps.tile([P, 1024], f32, tag=f"s{s_ctr[0]}",
                             name=f"s_{s_seq[0]}")

        def emit_scores_exp(qc, c, kt, eg, nkt):
            """Scores for (q-chunk qc, pair c, k-tile kt) -> exp into eg."""
            jl = kt - 4 * qc  # >=0 inside the diagonal quad
            diag = jl >= 0
            o = 128 * jl if diag else 0
            ps_s = s_tile()
            kf = qkfm[kt // 4][:, 4, :]
            qf = qkfm[qc]
            nc.tensor.matmul(
                ps_s[:, o:512],
                kf[0:64, ts(kt % 4, P)],
                qf[0:64, c, o:512],
                start=True, stop=True)
            nc.tensor.matmul(
                ps_s[:, 512 + o:1024],
                kf[64:128, ts(kt % 4, P)],
                qf[64:128, c, o:512],
                start=True, stop=True)
            if not diag:
                nc.scalar.activation(eg[:], ps_s[:], AF.Exp)
            else:
                nc.scalar.activation(
                    eg[:].rearrange("p (j q) -> p j q", q=512)[:, :, o:512],
                    ps_s[:].rearrange("p (j q) -> p j q", q=512)[:, :, o:512],
                    AF.Exp)
                # causal triangle at the diagonal 128-col block
                nc.gpsimd.affine_select(
                    eg[:].rearrange("p (j q) -> p j q", q=512)[:, :, o:o + 128],
                    eg[:].rearrange("p (j q) -> p j q", q=512)[:, :, o:o + 128],
                    pattern=[[0, 2], [1, 128]],
                    compare_op=OP.is_ge,
                    fill=0.0,
                    base=0,
                    channel_multiplier=-1)

        # ---------------- phase 1: qkv + norm + rope + transpose ----------
        with ExitStack() as p1:
            wpool = p1.enter_context(tc.tile_pool(name="wqkv", bufs=1))
            tpool = p1.enter_context(tc.tile_pool(name="tabs", bufs=1))
            XB = 4            # x-group prefetch depth (fat single-run
            # descriptors complete in ~2.4us, so in-loop issues no longer
            # starve the XBAR transposes via the completion ring)
            xpool = p1.enter_context(tc.tile_pool(name="xg", bufs=XB))
            # bufs=4: four token tiles in flight so each tile's serial
            # rmsnorm/rope chain + XBAR transpose DMA overlaps the next
            # tiles' matmuls
            wkk = p1.enter_context(tc.tile_pool(name="qkvwork", bufs=4))
            qkv_ps = p1.enter_context(
                tc.tile_pool(name="qkvpsum", bufs=2, space="PSUM"))

            # ALL input loads are issued up front as pure (wait-free) DMAs
            # so nothing with a runtime dependency (the XBAR transposes)
            # ever head-of-line-blocks a load on the in-order sync queue
            xgs = {}

            def issue_xg(g, split=1):
                xgs[g] = xpool.tile([P, DC, 256], bf16, tag="xg",
                                    name=f"xg{g}")
                src = xT[:, g]
                step = DC // split
                for s in range(split):
                    sl = slice(s * step, (s + 1) * step)
                    nc.sync.dma_start(xgs[g][:, sl], src[:, sl])

            # tile-0's inputs are issued interleaved in dc-consumption
            # order (x chunk s feeds dc 4s..4s+3, which read wq tile s)
            # so the first qkv chain never outruns the DMA stream
            wq_sb = [wpool.tile([P, 4, 768], bf16, name=f"wq{i}")
                     for i in range(4)]
            xgs[0] = xpool.tile([P, DC, 256], bf16, tag="xg", name="xg0")
            x0src = xT[:, 0]
            for s in range(4):
                sl = slice(4 * s, 4 * s + 4)
                nc.sync.dma_start(xgs[0][:, sl], x0src[:, sl])
                for j in range(4):
                    nc.sync.dma_start(wq_sb[s][:, j:j + 1],
                                      wqkv[:, 4 * s + j:4 * s + j + 1, :])
            issue_xg(1)

            tq = tpool.tile([P, NT, 4, 32], bf16)
            nc.sync.dma_start(tq[:], tabq)
            tk = tpool.tile([P, NT, 4, 32], bf16)
            nc.sync.dma_start(tk[:], tabk)
            for g in range(2, XB):
                issue_xg(g)

            # prefuse schedule: 4 units after each token tile from tile 8
            # on (their q/k XBAR chunks are long since landed; ScalarE has
            # slack under the qkv matmul stream).  qc0's 16 units first,
            # then qc1 pairs 0/1.
            prefuse = ([(0, c, j, 4 * c + j) for c in range(4)
                        for j in range(4)] +
                       [(1, c, j, 16 + 8 * c + j) for c in range(2)
                        for j in range(8)])
            pf = [0]

            def emit_prefuse(k):
                for _ in range(k):
                    if pf[0] < len(prefuse):
                        qc, c, j, idx = prefuse[pf[0]]
                        pf[0] += 1
                        emit_scores_exp(qc, c, j, eg0[idx][:], 4 * qc + 4)

            for g in range(2 * NG):
                if 1 <= g and g + XB - 1 < 2 * NG:
                    issue_xg(g + XB - 1)
                xg = xgs[g]
                for lt in range(2):
                    tt = g * 2 + lt
                    ps = qkv_ps.tile([P, 768], f32, tag="qkv")
                    for dc in range(DC):
                        lhsT = xg[:, dc, ts(lt, P)]
                        wsl = wq_sb[dc // 4][:, dc % 4]
                        nc.tensor.matmul(ps[:, 0:512], lhsT, wsl[:, 0:512],
                                         start=(dc == 0), stop=(dc == DC - 1))
                        nc.tensor.matmul(ps[:, 512:768], lhsT, wsl[:, 512:768],
                                         start=(dc == 0), stop=(dc == DC - 1))
                    # --- rmsnorm: rs = exp(-0.5*ln(ms+eps)); Square/Ln/Exp
                    # all live in one ACT table set, so no table reloads ---
                    sq = wkk.tile([P, 640], f32, tag="sq")
                    nc.scalar.activation(sq[:], ps[:, 0:640], AF.Square)
                    ssq = wkk.tile([P, 10], f32, tag="ssq")
                    nc.vector.reduce_sum(
                        ssq[:], sq[:].rearrange("p (h d) -> p h d", d=64),
                        axis=mybir.AxisListType.X)
                    lg = wkk.tile([P, 10], f32, tag="lg")
                    nc.scalar.activation(lg[:], ssq[:], AF.Ln,
                                         bias=eps_t[:], scale=1.0 / 64)
                    rs = wkk.tile([P, 10], f32, tag="rs")
                    nc.scalar.activation(rs[:], lg[:], AF.Exp, scale=-0.5)
                    qn = wkk.tile([P, 512], bf16, tag="qn")
                    nc.vector.tensor_tensor(
                        qn[:].rearrange("p (h d) -> p h d", d=64),
                        ps[:, 0:512].rearrange("p (h d) -> p h d", d=64),
                        rs[:, 0:8, None].to_broadcast([P, 8, 64]), OP.mult)
                    kn = wkk.tile([P, 128], bf16, tag="kn")
                    nc.vector.tensor_tensor(
                        kn[:].rearrange("p (h d) -> p h d", d=64),
                        ps[:, 512:640].rearrange("p (h d) -> p h d", d=64),
                        rs[:, 8:10, None].to_broadcast([P, 2, 64]), OP.mult)
                    # --- v copy on the DVE (ones cols at 64/129) ---
                    nc.vector.tensor_copy(
                        vsb[tt][:, 0:130].rearrange(
                            "p (j q) -> p j q", q=65)[:, :, 0:64],
                        ps[:, 640:768].rearrange("p (j q) -> p j q", q=64))
                    # --- rope: 3 DVE ops per tensor via host-folded tables
                    # tab rows are [A, B, C, -D]; viewed as [P, 2, 2, 32] the
                    # pairs are (A,C) and (B,-D), so
                    # dv = t1*(A,C) - t2*(B,-D) = (t1*A - t2*B | t1*C + t2*D)
                    qkr = wkk.tile([P, 640], bf16, tag="qkr")
                    for (src, dsl, tab, nh) in ((qn, slice(0, 512), tq, 8),
                                                (kn, slice(512, 640), tk, 2)):
                        sv = src[:].rearrange("p (h d) -> p h d", d=64)
                        dv = qkr[:, dsl].rearrange("p (h two f) -> p h two f",
                                                   two=2, f=32)
                        tabv = tab[:, tt].rearrange("p (g two) f -> p two g f",
                                                    two=2)
                        t1 = sv[:, :, None, 0:32].to_broadcast([P, nh, 2, 32])
                        t2 = sv[:, :, None, 32:64].to_broadcast([P, nh, 2, 32])
                        AC = tabv[:, 0:1, :, :].to_broadcast([P, nh, 2, 32])
                        BD = tabv[:, 1:2, :, :].to_broadcast([P, nh, 2, 32])
                        u13 = wkk.tile([P, nh, 2, 32], bf16, tag=f"u13_{nh}")
                        u24 = wkk.tile([P, nh, 2, 32], bf16, tag=f"u24_{nh}")
                        nc.vector.tensor_tensor(u13[:], t1, AC, OP.mult)
                        nc.vector.tensor_tensor(u24[:], t2, BD, OP.mult)
                        nc.vector.tensor_tensor(dv, u13[:], u24[:],
                                                OP.subtract)
                    # --- transpose to feature-major via the DMA XBAR:
                    # out[p, g, t] = qkr[t, 128g + p], one dma for q AND k
                    # (keeps the in-order sync queue's descriptor-gen time
                    # per tile low enough to track the rope stream) ---
                    nc.sync.dma_start_transpose(
                        qkfm[tt // 4][:, :, ts(tt % 4, P)], qkr[:])
                    if tt >= 8:
                        emit_prefuse(4)
            emit_prefuse(len(prefuse))  # safety: should be drained already

        # ---------------- phase 2: attention + out projection ------------
        wopool = ctx.enter_context(tc.tile_pool(name="wo", bufs=1))
        wo_sb = wopool.tile([P, 4, D_MODEL], bf16, name="wo_sb")
        nc.sync.dma_start(wo_sb[:], wo)
        with ExitStack() as p2:
            epool = p2.enter_context(tc.tile_pool(name="exp", bufs=7))
            npool = p2.enter_context(tc.tile_pool(name="nrm", bufs=2))
            opool = p2.enter_context(tc.tile_pool(name="osb", bufs=3))
            y_ps = p2.enter_context(
                tc.tile_pool(name="ypsum", bufs=1, space="PSUM"))
            y_ctr = [0]

            y_seq = [0]

            def y_tile():
                y_ctr[0] ^= 1
                y_seq[0] += 1
                return y_ps.tile([65, 1024], f32, tag=f"y{y_ctr[0]}",
                                 name=f"y_{y_seq[0]}")

            pending = []          # (token tile, output half) groups
            flush_ctr = [0]
            OG_INLINE = True      # inline out-proj at sustainable cadence

            def emit_ogroup(cast_on_scalar=False):
                t, og = pending.pop(0)
                ps_o = s_tile()
                for oc2 in range(2):
                    for yc in range(4):
                        nc.tensor.matmul(
                            ps_o[:, ts(oc2, 512)],
                            yfm[yc][:, ts(t, P)],
                            wo_sb[:, yc, 1024 * og + 512 * oc2:
                                  1024 * og + 512 * (oc2 + 1)],
                            start=(yc == 0), stop=(yc == 3))
                ob = opool.tile([P, 1024], f16, tag="ob")
                if cast_on_scalar:
                    # drain: exps are over, ScalarE is free, and the DVE is
                    # busy with the final normalize chains
                    nc.scalar.activation(ob[:], ps_o[:], AF.Copy)
                else:
                    nc.vector.tensor_copy(ob[:], ps_o[:])
                nc.sync.dma_start(out[ts(t, P), ts(og, 1024)], ob[:])

            def emit_pv(kt, eg, o, ps_y, nkt):
                nc.tensor.matmul(
                    ps_y[:, o:512], vsb[kt][:, 0:65], eg[:, o:512],
                    start=(kt == 0), stop=(kt == nkt - 1))
                nc.tensor.matmul(
                    ps_y[:, 512 + o:1024], vsb[kt][:, 65:130],
                    eg[:, 512 + o:1024],
                    start=(kt == 0), stop=(kt == nkt - 1))

            def normalize(ps_y, c, qc):
                # 1/den via DVE recip + GpSimd partition broadcast
                # (recip can't read PSUM; vector copy bounces row 64)
                draw = npool.tile([1, 1024], f32, tag="draw")
                nc.vector.tensor_copy(draw[0:1, :], ps_y[64:65, :])
                rec = npool.tile([1, 1024], f32, tag="rec")
                nc.vector.reciprocal_approx_fast(rec[0:1, :], draw[0:1, :])
                rexp = npool.tile([64, 1024], f32, tag="rexp")
                nc.gpsimd.partition_broadcast(rexp[:], rec[0:1, :],
                                              channels=64)
                nc.vector.tensor_tensor(yfm[c][0:64, ts(qc, 512)],
                                        ps_y[0:64, 0:512],
                                        rexp[:, 0:512], OP.mult)
                nc.vector.tensor_tensor(yfm[c][64:128, ts(qc, 512)],
                                        ps_y[0:64, 512:1024],
                                        rexp[:, 512:1024], OP.mult)

            # `held` carries the not-yet-emitted PV (+ pair finalizer) of the
            # previous k-tile ACROSS pair boundaries, so scores/exp of the
            # next pair keep both engines fed while the last PV of the
            # previous pair waits on its exp semaphore.
            held = []   # FIFO of (pv_fn, final_fn or None); depth-2 skew

            # --- prefused chunks: qc0 pairs 0/1 emitted directly; qc0
            # pairs 2/3 and qc1 pairs 0/1 seeded into `held` so their PVs
            # (gated by the y-ring normalize chains) interleave with the
            # live scores stream instead of blocking the in-order tensor
            # queue.  y tiles are allocated lazily at pop time so the ring
            # reuse order matches execution order.
            def avalanche(c):
                ps_y = y_tile()
                for j in range(4):
                    emit_pv(j, eg0[4 * c + j][:], 128 * j, ps_y, 4)
                normalize(ps_y, c, 0)

            avalanche(0)
            avalanche(1)

            def seed_pair(qc, c, base):
                box = []
                nkt = 4 * qc + 4
                h = nkt // 2

                def run(j0, j1):
                    if not box:
                        box.append(y_tile())
                    for j in range(j0, j1):
                        jl = j - 4 * qc
                        emit_pv(j, eg0[base + j][:],
                                128 * jl if jl >= 0 else 0, box[0], nkt)

                held.append((lambda: run(0, h), None))
                held.append((lambda: run(h, nkt),
                             lambda: normalize(box[0], c, qc)))

            seed_pair(0, 2, 8)
            seed_pair(0, 3, 12)
            seed_pair(1, 0, 16)
            seed_pair(1, 1, 24)
            for tl in range(4):
                for og in range(2):
                    pending.append((tl, og))

            def flush_held():
                if not held:
                    return
                pv_fn, final_fn = held.pop(0)
                pv_fn()
                if final_fn is not None:
                    final_fn()
                flush_ctr[0] += 1
                # don't queue an ogroup cast on the DVE right before a
                # pair finalize: the cast would delay the normalize mults
                # that release the y-ring for the pair after next
                imminent = held and held[0][1] is not None
                if (OG_INLINE and pending and not imminent
                        and flush_ctr[0] >= 3 and flush_ctr[0] % 4 == 0):
                    emit_ogroup()

            for qc in range(1, QC):
                for c in (range(2, 4) if qc == 1 else range(4)):
                    nkt = 4 * qc + 4
                    ybox = []

                    def get_y(ybox=ybox):
                        if not ybox:
                            ybox.append(y_tile())
                        return ybox[0]

                    for kt in range(nkt):
                        jl = kt - 4 * qc
                        diag = jl >= 0
                        o = 128 * jl if diag else 0
                        eg = epool.tile([P, 1024], bf16, tag="eg")
                        emit_scores_exp(qc, c, kt, eg[:], nkt)
                        if len(held) >= 2:
                            flush_held()
                        is_last = kt == nkt - 1
                        held.append((
                            lambda kt=kt, eg=eg, o=o, g=get_y, nkt=nkt:
                                emit_pv(kt, eg[:], o, g(), nkt),
                            (lambda g=get_y, c=c, qc=qc:
                                normalize(g(), c, qc)) if is_last else None))
                # drain the seed backlog so the held skew returns to 2
                # before the next chunk's stream begins
                while len(held) > 2:
                    flush_held()
                # queue this q-chunk's out-projection groups (flushed in
                # the drain)
                for tl in range(4):
                    for og in range(2):
                        pending.append((4 * qc + tl, og))
            while held:
                flush_held()
            while pending:
                emit_ogroup(cast_on_scalar=True)

    # Force a single ScalarE activation table: natural_log_exp_and_others
    # holds every func this kernel uses (Exp, Ln, Square, Copy).  The
    # greedy per-function chooser would otherwise ping-pong between the
    # exp and natural-log sets (1.28us per ACT_TABLE_LOAD).  Emptying the
    # other sets (names/indices preserved, so act_func_set_id stays valid)
    # makes the covering analysis land every activation on the one set.
    from concourse import hw_specs
    tables = hw_specs.get_activation_tables(nc.m.arch)
    for name, funcs in tables.items():
        if name != "natural_log_exp_and_others":
            funcs.clear()
    nc.compile()
    return nc


def _rope_tables(pos, norm_w, scale):
    """Build [P, NT, 4, 32] tables A,B,C,D for out1 = t1*A - t2*B,
    out2 = t1*C + t2*D (NeoX rope with folded norm weight + score scale)."""
    n_tok = pos.shape[0]
    f = np.arange(0, D_HEAD, 2, dtype=np.float64) / D_HEAD
    inv_freq = 1.0 / (ROPE_BASE ** f)                       # [32]
    ang = pos.astype(np.float64)[:, None] * inv_freq[None, :]  # [n, 32]
    cos, sin = np.cos(ang), np.sin(ang)
    w1 = norm_w[:32].astype(np.float64)
    w2 = norm_w[32:].astype(np.float64)
    A = cos * w1 * scale
    Bt = sin * w2 * scale
    C = sin * w1 * scale
    D = cos * w2 * scale
    # D negated: the kernel computes t1*(A,C) - t2*(B,-D) in two fused ops
    tab = np.stack([A, Bt, C, -D], axis=1).astype(np.float32)  # [n, 4, 32]
    return np.ascontiguousarray(
        tab.reshape(n_tok // P, P, 4, 32).transpose(1, 0, 2, 3))


def make_in_maps(x, pos, qkv_w, out_w, q_norm_w, k_norm_w, n_tok=N):
    import ml_dtypes
    bf16 = ml_dtypes.bfloat16

    scale = D_HEAD ** -0.5
    tabq = _rope_tables(pos, q_norm_w, scale).astype(bf16)
    tabk = _rope_tables(pos, k_norm_w, 1.0).astype(bf16)
    wq_all = qkv_w[0:H_Q * D_HEAD].reshape(H_Q, D_HEAD, D_MODEL)
    wk_all = qkv_w[H_Q * D_HEAD:(H_Q + H_KV) * D_HEAD].reshape(
        H_KV, D_HEAD, D_MODEL)
    wv_all = qkv_w[(H_Q + H_KV) * D_HEAD:].reshape(H_KV, D_HEAD, D_MODEL)
    wo_all = out_w.reshape(D_MODEL, H_Q, D_HEAD)

    in_maps = []
    for c in range(NCORES):
        b, hg = divmod(c, 4)
        # head order [0,4,1,5,2,6,3,7]: pair i = (head on kv0, head on kv1)
        # so the kernel's two 64-partition row groups use kfm's two halves
        # directly (no partition-swapped K copy)
        heads = [8 * hg + (i % 2) * 4 + i // 2 for i in range(8)]
        kvs = [2 * hg, 2 * hg + 1]
        wsel = np.concatenate([
            wq_all[heads].reshape(512, D_MODEL),
            wk_all[kvs].reshape(128, D_MODEL),
            wv_all[kvs].reshape(128, D_MODEL)], axis=0)    # [768, D]
        # pre-tiled: xT[p, g, o, t], wqkv[p, o, r], wo[p, o, d] — see the
        # dram_tensor declarations in build_nc
        xt = np.ascontiguousarray(
            x[b].T.reshape(16, P, n_tok // 256, 256).transpose(1, 2, 0, 3)
        ).astype(bf16)
        wq_t = np.ascontiguousarray(
            wsel.T.reshape(16, P, 768).transpose(1, 0, 2)).astype(bf16)
        wo_t = np.ascontiguousarray(
            wo_all[:, heads].reshape(D_MODEL, 512).T
            .reshape(4, P, D_MODEL).transpose(1, 0, 2)).astype(bf16)
        in_maps.append({
            "xT": xt,
            "wqkv": wq_t,
            "wo": wo_t,
            "tabq": tabq,
            "tabk": tabk,
        })
    return in_maps


def _reference_host(x, mask, pos, qkv_w, out_w, q_norm_w, k_norm_w):
    """Pure-numpy fallback, used only if the mask is not causal."""
    xx = x.astype(np.float64)
    qkv = xx @ qkv_w.T.astype(np.float64)
    Bsz, Nl, _ = x.shape
    qkv = qkv.reshape(Bsz, Nl, H_Q + 2 * H_KV, D_HEAD).transpose(0, 2, 1, 3)
    q, k, v = (qkv[:, :H_Q], qkv[:, H_Q:H_Q + H_KV], qkv[:, H_Q + H_KV:])

    def rms(t, w):
        var = np.mean(t * t, axis=-1, keepdims=True)
        return t / np.sqrt(var + EPS) * w

    def rope(t):
        f = np.arange(0, D_HEAD, 2) / D_HEAD
        inv = 1.0 / (ROPE_BASE ** f)
        ang = pos.astype(np.float64)[:, None] * inv[None, :]
        cs, sn = np.cos(ang), np.sin(ang)
        t1, t2 = t[..., :32], t[..., 32:]
        return np.concatenate([t1 * cs - t2 * sn, t1 * sn + t2 * cs], axis=-1)

    q, k = rope(rms(q, q_norm_w)), rope(rms(k, k_norm_w))
    qg = q.reshape(Bsz, H_KV, 4, Nl, D_HEAD)
    sc = np.einsum("bhgnd,bhmd->bhgnm", qg, k) * (D_HEAD ** -0.5)
    sc = np.where(mask[None, None, None], -np.inf, sc)
    sc -= sc.max(axis=-1, keepdims=True)
    p = np.exp(sc)
    p /= p.sum(axis=-1, keepdims=True)
    y = np.einsum("bhgnm,bhmd->bhgnd", p, v)
    y = y.reshape(Bsz, H_Q, Nl, D_HEAD).transpose(0, 2, 1, 3).reshape(
        Bsz, Nl, D_MODEL)
    return (y @ out_w.T.astype(np.float64)).astype(np.float32)


_NC_CACHE = {}


def run_on_device(in_maps, n_tok=N, trace=False, trace_kwargs=None):
    import sys
    for p in ("/opt/trn_rl_repo",):
        if p not in sys.path:
            sys.path.insert(0, p)
    from concourse.bass_utils import run_bass_kernel_spmd

    key = n_tok
    if key not in _NC_CACHE:
        _NC_CACHE[key] = build_nc(n_tok)
    nc = _NC_CACHE[key]
    return run_bass_kernel_spmd(
        nc, in_maps, list(range(len(in_maps))), trace=trace,
        **(trace_kwargs or {}))


def kernel(x, mask, pos, qkv_w, out_w, q_norm_w, k_norm_w):
    x = np.asarray(x, dtype=np.float32)
    mask = np.asarray(mask)
    pos = np.asarray(pos)
    causal = bool(
        np.array_equal(mask,
                       np.triu(np.ones((N, N), dtype=bool), k=1)))
    if not causal:
        return _reference_host(x, mask, pos, np.asarray(qkv_w),
                               np.asarray(out_w), np.asarray(q_norm_w),
                               np.asarray(k_norm_w))
    in_maps = make_in_maps(x, pos, np.asarray(qkv_w, dtype=np.float32),
                           np.asarray(out_w, dtype=np.float32),
                           np.asarray(q_norm_w, dtype=np.float32),
                           np.asarray(k_norm_w, dtype=np.float32))
    res = run_on_device(in_maps)
    outs = [r["out"].astype(np.float32) for r in res.results]
    full = np.empty((B, N, D_MODEL), dtype=np.float32)
    for b in range(B):
        full[b] = outs[4 * b] + outs[4 * b + 1] + outs[4 * b + 2] + outs[4 * b + 3]
    return full
